# revision 7
# baseline (speedup 1.0000x reference)
"""BitLinear-1.58b Trainium2 kernel.

Computation (see BitLinear reference):
  scale = clip(mean(|W|), eps)                 (scalar)
  qw    = clip(round(W/scale), -1, 1)          (ternary)
  gamma = clip(max|x| per token, eps)
  qx    = clip(round(x * 128/gamma), -128, 127)
  y     = (qx @ qw^T + bias) * scale*gamma/128

Distribution: tokens (B*S = 8192) are data-parallel sharded 1024 per core
across 8 cores; the weight (and its quantization work) is replicated.
The scalar `scale` is computed on host before sharding (it is a global
mean over the full weight).  The weight is passed to each core already
transposed ([DIN, DOUT]) -- a pure host-side layout transformation -- so
the device quantizes it directly into the matmul-ready K-major layout.

All quantized values (qx in [-128,127], qw in {-1,0,1}) are exactly
representable in bf16, and every partial dot product is an integer with
magnitude <= 2048*128 < 2^24, so a bf16 matmul with fp32 PSUM
accumulation reproduces the reference arithmetic exactly.  Rounding uses
the fp32 magic-number trick (v + 1.5*2^23 - 1.5*2^23) which matches
round-half-to-even exactly for |v| < 2^22.
"""

import numpy as np

import concourse.bass as bass
import concourse.mybir as mybir
import concourse.tile as tile
from concourse import bacc
from concourse.bass_utils import run_bass_kernel_spmd

P = 128
DIN = 2048
DOUT = 2048
N_CORES = 8
TOK = 1024  # tokens per core
KT = DIN // P  # 16 k-tiles
MT = TOK // P  # 8 m-tiles
NW = 512  # matmul moving free dim (one PSUM bank)
NB = DOUT // NW  # 4 n-blocks

F32 = mybir.dt.float32
BF16 = mybir.dt.bfloat16
ALU = mybir.AluOpType
AFT = mybir.ActivationFunctionType

MAGIC = 12582912.0  # 1.5 * 2^23: fp32 round-to-nearest-even magic constant
EPS = 1e-5
Q = 128.0

_CACHE: dict = {}

# test harness hooks (set by test.py; harmless defaults for grading)
TRACE = False
LAST_RESULTS = None


def _build():
    nc = bacc.Bacc("TRN2", target_bir_lowering=False, debug=False)

    x_d = nc.dram_tensor("x", [TOK, DIN], F32, kind="ExternalInput")
    wt_d = nc.dram_tensor("wt", [DIN, DOUT], F32, kind="ExternalInput")
    bias_d = nc.dram_tensor("biasrep", [P, DOUT], F32, kind="ExternalInput")
    # consts[:, 0] = 1/scale, consts[:, 1] = scale/128  (replicated per partition)
    consts_d = nc.dram_tensor("consts", [P, 2], F32, kind="ExternalInput")
    y_d = nc.dram_tensor("y", [TOK, DOUT], F32, kind="ExternalOutput")

    with tile.TileContext(nc) as tc:
        with (
            tc.tile_pool(name="const", bufs=1) as cpool,
            tc.tile_pool(name="wq", bufs=1) as wq_pool,
            tc.tile_pool(name="qxt", bufs=1) as qxt_pool,
            tc.tile_pool(name="xstage", bufs=3) as xstage,
            tc.tile_pool(name="wstage", bufs=4) as wstage,
            tc.tile_pool(name="xtmp", bufs=3) as xtmp,
            tc.tile_pool(name="wtmp", bufs=4) as wtmp,
            tc.tile_pool(name="qn", bufs=2) as qn_pool,
            tc.tile_pool(name="outp", bufs=4) as outp,
            tc.tile_pool(name="small", bufs=4) as small,
            tc.tile_pool(name="psum", bufs=8, space="PSUM") as psum_pool,
        ):
            biasrep = cpool.tile([P, DOUT], F32, tag="biasrep")
            nc.sync.dma_start(biasrep[:], bias_d[:])
            consts = cpool.tile([P, 2], F32, tag="consts")
            nc.sync.dma_start(consts[:], consts_d[:])
            inv_scale = consts[:, 0:1]
            s128 = consts[:, 1:2]

            # ---- weight quantization, n-panel major so matmul chains can
            # start as soon as panel 0 is ready ----
            qwt = [[None] * KT for _ in range(NB)]

            def quant_w_panel(n):
                for k in range(KT):
                    wch = wstage.tile([P, NW], F32, tag="wstage")
                    nc.sync.dma_start(
                        wch[:], wt_d[k * P : (k + 1) * P, n * NW : (n + 1) * NW]
                    )
                    # t = w * (1/scale) + MAGIC      (ScalarE)
                    t = wtmp.tile([P, NW], F32, tag="wtmp")
                    nc.scalar.activation(
                        t[:], wch[:], AFT.Copy, bias=MAGIC, scale=inv_scale
                    )
                    # u = min(t, MAGIC+1) - MAGIC    -> round(w/scale) clipped above
                    u = wtmp.tile([P, NW], F32, tag="wtmp")
                    nc.vector.tensor_scalar(
                        u[:], t[:], MAGIC + 1.0, MAGIC, op0=ALU.min, op1=ALU.subtract
                    )
                    # qw = max(u, -1) -> bf16 (exact)
                    qw_k = wq_pool.tile([P, NW], BF16, tag=f"qw_{n}_{k}")
                    nc.vector.tensor_scalar_max(qw_k[:], u[:], -1.0)
                    qwt[n][k] = qw_k

            quant_w_panel(0)

            # ---- activation quantization per 128-token tile ----
            qxt = [None] * MT
            mscale = [None] * MT

            def quant_x_tile(m):
                xch = xstage.tile([P, DIN], F32, tag="xstage")
                nc.sync.dma_start(xch[:], x_d[m * P : (m + 1) * P, :])
                g0 = small.tile([P, 1], F32, tag="g0")
                nc.vector.tensor_reduce(
                    g0[:],
                    xch[:],
                    axis=mybir.AxisListType.X,
                    op=ALU.max,
                    apply_absolute_value=True,
                )
                gamma = small.tile([P, 1], F32, tag="gamma")
                nc.vector.tensor_scalar_max(gamma[:], g0[:], EPS)
                # r = 128/gamma == 1/(gamma/128); gamma/128 is exact, so a
                # correctly-rounded reciprocal reproduces fl(128/gamma)
                g128 = small.tile([P, 1], F32, tag="g128")
                nc.vector.tensor_scalar_mul(g128[:], gamma[:], 1.0 / Q)
                r = small.tile([P, 1], F32, tag="r")
                nc.vector.reciprocal(r[:], g128[:])
                ms = cpool.tile([P, 1], F32, tag=f"ms_{m}")
                nc.vector.tensor_scalar_mul(ms[:], gamma[:], s128)
                mscale[m] = ms
                # t = x*r + MAGIC ; u = min(t, MAGIC+127) - MAGIC ; q = max(u, -128)
                t = xtmp.tile([P, DIN], F32, tag="xtmp")
                nc.vector.tensor_scalar(
                    t[:], xch[:], r[:], MAGIC, op0=ALU.mult, op1=ALU.add
                )
                u = xtmp.tile([P, DIN], F32, tag="xtmp")
                nc.vector.tensor_scalar(
                    u[:], t[:], MAGIC + (Q - 1.0), MAGIC, op0=ALU.min, op1=ALU.subtract
                )
                qxn = qn_pool.tile([P, DIN], BF16, tag="qxn")
                nc.vector.tensor_scalar_max(qxn[:], u[:], -Q)
                # transpose to [DIN, tok] layout: qt[p, k, j] = qxn[j, k*128+p]
                qt = qxt_pool.tile([P, KT, P], BF16, tag=f"qxt_{m}")
                nc.sync.dma_start_transpose(qt[:], qxn[:])
                qxt[m] = qt

            for m in range(MT):
                quant_x_tile(m)
            for n in range(1, NB):
                quant_w_panel(n)

            # ---- matmul + epilogue ----
            for n in range(NB):
                for m in range(MT):
                    ps = psum_pool.tile([P, NW], F32, tag="ps")
                    for k in range(KT):
                        nc.tensor.matmul(
                            ps[:],
                            qxt[m][:, k, :],
                            qwt[n][k][:],
                            start=(k == 0),
                            stop=(k == KT - 1),
                        )
                    s1 = outp.tile([P, NW], F32, tag="s1")
                    nc.vector.tensor_tensor(
                        s1[:], ps[:], biasrep[:, n * NW : (n + 1) * NW], op=ALU.add
                    )
                    o = outp.tile([P, NW], F32, tag="o")
                    nc.scalar.activation(
                        o[:], s1[:], AFT.Copy, bias=0.0, scale=mscale[m][:]
                    )
                    nc.sync.dma_start(
                        y_d[m * P : (m + 1) * P, n * NW : (n + 1) * NW], o[:]
                    )
    nc.compile()
    return nc


def kernel(x: np.ndarray, weight: np.ndarray, bias: np.ndarray) -> np.ndarray:
    global LAST_RESULTS
    B, S, _ = x.shape

    x2 = np.ascontiguousarray(x.reshape(B * S, DIN), dtype=np.float32)
    wt = np.ascontiguousarray(weight.T.astype(np.float32, copy=False))
    biasrep = np.ascontiguousarray(
        np.broadcast_to(bias.astype(np.float32, copy=False), (P, DOUT))
    )
    # global scalar: computed on host before sharding (see sharding note)
    scale = np.float32(max(np.mean(np.abs(weight), dtype=np.float64), EPS))
    inv_scale = np.float32(1.0 / np.float64(scale))
    s128 = np.float32(scale) / np.float32(Q)  # exact (power-of-two divide)
    consts = np.empty((P, 2), dtype=np.float32)
    consts[:, 0] = inv_scale
    consts[:, 1] = s128

    if "nc" not in _CACHE:
        _CACHE["nc"] = _build()
    nc = _CACHE["nc"]

    in_maps = [
        {
            "x": x2[i * TOK : (i + 1) * TOK],
            "wt": wt,
            "biasrep": biasrep,
            "consts": consts,
        }
        for i in range(N_CORES)
    ]
    res = run_bass_kernel_spmd(
        nc,
        in_maps,
        list(range(N_CORES)),
        trace=TRACE,
        trace_cores=list(range(N_CORES)) if TRACE else None,
    )
    LAST_RESULTS = res
    out = np.concatenate([res.results[i]["y"] for i in range(N_CORES)], axis=0)
    return np.ascontiguousarray(out.reshape(B, S, DOUT).astype(np.float32, copy=False))
